# revision 18
# baseline (speedup 1.0000x reference)
"""NeuralODE (nn_NeuralODE_36807869727439) Trainium2 Bass kernel, 8 NeuronCores.

Math: 26 Euler steps of z += h * (tanh(z@W1 + b1 + t*u) @ W2 + b2), with
B=256, D=2048, H=4096 and the step grid derived from the input t exactly as
the reference does.

Distribution scheme (tensor-parallel over H, one AllGather per step):
  * Track p = z @ W1 (+ accumulated bias drift) instead of z.  With
    G = W2 @ W1 (host-precomputed, fp16) the recurrence is
        a_k = tanh(p_k + c_k),   p_{k+1} = p_k + h_k * (a_k @ G)
    where c_k = b1 + t_k*u + (sum_{j<k} h_j) * (b2@W1) is host-precomputed.
    Since H = 2D, a_k @ G has exactly the same FLOPs as the two original
    GEMMs per step.
  * Core i holds column shard G[:, 512i:512(i+1)] (fp16, 4MB, SBUF-resident)
    and the state shard p[:, H_i] in T-layout [512, 256] fp32.
  * Per step, each core computes ha = h*tanh(p+c) (fp16, 128KB), and one
    8-rank AllGather assembles ha_full [4096, B] for the GEMM rhs.
    The batch is split in two halves -> two independent software pipelines,
    so each half's GEMM/tanh hides under the other half's AllGather.
  * z_final = z0 + (sum_k h_k a_k) @ W2 + (sum h_k) b2 is linear in the a_k:
    each core accumulates S = sum h_k a_k for its H-shard (fp32, DVE), runs
    one fp32 GEMM against W2[H_i, :] at the end, and the host sums the eight
    [2048, 256] partials (no z exchange, no final collective).
"""
import math
import sys

import numpy as np

if "/opt/trn_rl_repo" not in sys.path:
    sys.path.insert(0, "/opt/trn_rl_repo")

B = 256
D = 2048
H = 4096
N_CORES = 8
H_LOC = H // N_CORES          # 512
H_MAX = 0.05                  # ODEsolver_Euler default max step
KCH = H // 128                # 32 contraction chunks
MT = H_LOC // 128             # 4 m-tiles per core


def _compute_schedule(t):
    """Mirror reference._euler_solve stepping exactly (fp64 interval math,
    fp32 h and fp32 accumulated t)."""
    t64 = np.asarray(t, dtype=np.float64)
    sched = []
    for i in range(t64.shape[0] - 1):
        t0, t1 = t64[i], t64[i + 1]
        n = int(math.ceil(abs(t1 - t0) / H_MAX))
        if n == 0:
            continue
        h = np.float32((t1 - t0) / n)
        tc = np.float32(t0)
        for _ in range(n):
            tc = np.float32(tc + h)
            sched.append((float(h), float(tc)))
    return sched


def _host_prepare(z0, W1, b1, u, W2, b2, sched):
    f32, f16, f64 = np.float32, np.float16, np.float64
    nsteps = len(sched)
    G16 = ((W2.astype(f64) @ W1.astype(f64)) / 16.0).astype(f16)  # [H, H]/16
    b2W1 = (b2.astype(f64) @ W1.astype(f64)).astype(f32)         # [H]
    p0 = z0.astype(f32) @ W1.astype(f32)                         # [B, H]
    hs = np.array([h for h, _ in sched], dtype=f32)
    cumh = np.concatenate([[0.0], np.cumsum(hs.astype(f64))[:-1]]).astype(f32)
    ts = np.array([tc for _, tc in sched], dtype=f32)
    cbias = (b1[None, :].astype(f32)
             + ts[:, None] * u[None, :].astype(f32)
             + cumh[:, None] * b2W1[None, :])                    # [nsteps, H]
    # step-0 gathered payload host-computed: the first step needs no AllGather
    ha0 = (16.0 * hs[0] * np.tanh(p0 + cbias[0])).astype(f16)   # 16*h0*a0
    ha0_dev = np.ascontiguousarray(
        ha0.T.reshape(KCH, 128, B).transpose(1, 0, 2))           # [128, KCH, B]

    in_maps = []
    for i in range(N_CORES):
        hlo = H_LOC * i
        Gc = G16[:, hlo:hlo + H_LOC]
        Gc_dev = np.ascontiguousarray(
            Gc.reshape(KCH, 128, H_LOC).transpose(1, 0, 2))      # [128, 32, 512]
        p0T = p0[:, hlo:hlo + H_LOC].T
        p0_dev = np.ascontiguousarray(p0T.reshape(MT, 128, B))   # [4, 128, 256]
        cb = cbias[:, hlo:hlo + H_LOC]
        cb_dev = np.ascontiguousarray(
            cb.reshape(nsteps, MT, 128).transpose(2, 0, 1).reshape(128, nsteps * MT))
        W2r = W2[hlo:hlo + H_LOC, :].astype(f32)
        W2r_dev = np.ascontiguousarray(W2r.reshape(MT, 128, D))  # [4, 128, 2048]
        in_maps.append({
            "g_in": Gc_dev,
            "p0_in": p0_dev,
            "cb_in": cb_dev,
            "w2_in": W2r_dev,
            "ha0_in": ha0_dev,
        })
    return in_maps


def _build_program(sched, split=2, haf_group=8):
    import concourse.bacc as bacc
    import concourse.mybir as mybir
    import concourse.tile as tile

    nsteps = len(sched)
    nc = bacc.Bacc("TRN2", target_bir_lowering=False, debug=False,
                   num_devices=N_CORES)

    g_in = nc.dram_tensor("g_in", [128, KCH, H_LOC], mybir.dt.float16, kind="ExternalInput")
    p0_in = nc.dram_tensor("p0_in", [MT, 128, B], mybir.dt.float32, kind="ExternalInput")
    cb_in = nc.dram_tensor("cb_in", [128, nsteps * MT], mybir.dt.float32, kind="ExternalInput")
    w2_in = nc.dram_tensor("w2_in", [MT, 128, D], mybir.dt.float32r, kind="ExternalInput")
    ha0_in = nc.dram_tensor("ha0_in", [128, KCH, B], mybir.dt.float16, kind="ExternalInput")
    zf_out = nc.dram_tensor("zf_out", [D // 128, 128, B], mybir.dt.float32, kind="ExternalOutput")

    BS = B // split
    with tile.TileContext(nc) as tc:
        with (
            tc.tile_pool(name="sbuf", bufs=1) as pool,
            tc.tile_pool(name="psum", bufs=1, space="PSUM") as psum_pool,
            tc.tile_pool(name="dram", bufs=1, space="DRAM") as dram_pool,
        ):
            G_sb = pool.tile([128, KCH, H_LOC], mybir.dt.float16, tag="G_sb")
            nc.scalar.dma_start(G_sb[:, :KCH // 2, :], g_in[:, :KCH // 2, :])
            nc.sync.dma_start(G_sb[:, KCH // 2:, :], g_in[:, KCH // 2:, :])
            cb_sb = pool.tile([128, nsteps * MT], mybir.dt.float32, tag="cb_sb")
            nc.sync.dma_start(cb_sb[:], cb_in[:])
            p_sb = pool.tile([128, MT, B], mybir.dt.float32, tag="p_sb")
            for m in range(MT):
                nc.sync.dma_start(p_sb[:, m, :], p0_in[m])
            S_sb = pool.tile([128, MT, B], mybir.dt.float32, tag="S_sb")
            nc.vector.memset(S_sb[:], 0.0)

            def produce_ha(k, hx, m, ha_sb, ha8_sb, ag_i):
                cs = hx * BS
                h_k = sched[k][0]
                a_t = pool.tile([128, BS], mybir.dt.float32,
                                tag=f"a_t{hx}{m}", bufs=3, name=f"a_{k}_{hx}_{m}")
                nc.scalar.activation(
                    a_t[:], p_sb[:, m, cs:cs + BS],
                    mybir.ActivationFunctionType.Tanh,
                    bias=cb_sb[:, k * MT + m:k * MT + m + 1],
                )
                nc.vector.tensor_scalar_mul(ha_sb[:, m * BS:(m + 1) * BS], a_t[:],
                                            float(h_k))
                nc.vector.tensor_tensor(
                    S_sb[:, m, cs:cs + BS], S_sb[:, m, cs:cs + BS],
                    ha_sb[:, m * BS:(m + 1) * BS], mybir.AluOpType.add,
                )
                if ag_i is not None:
                    # fp8 wire copy at 16*h scale (G carries the 1/16)
                    nc.vector.tensor_scalar_mul(
                        ha8_sb[:, m * BS:(m + 1) * BS], a_t[:],
                        float(16.0 * h_k))
                    nc.sync.dma_start(
                        ag_i[m * 128:(m + 1) * 128, :],
                        ha8_sb[:, m * BS:(m + 1) * BS])

            def new_ha_buffers(k, hx, with_agi=True):
                ha_sb = pool.tile([128, MT * BS], mybir.dt.float16,
                                  tag=f"ha_sb{hx}", bufs=3, name=f"ha_{k}_{hx}")
                ha8_sb = None
                ag_i = None
                if with_agi:
                    ha8_sb = pool.tile([128, MT * BS], mybir.dt.float8e4,
                                       tag=f"ha8_sb{hx}", bufs=3,
                                       name=f"ha8_{k}_{hx}")
                    ag_i = dram_pool.tile([H_LOC, BS], mybir.dt.float8e4,
                                          tag=f"agi_{k}_{hx}", name=f"agi_{k}_{hx}")
                return ha_sb, ha8_sb, ag_i

            anchors = []
            haf0 = pool.tile([128, KCH, B], mybir.dt.float16, tag="hafz")
            nc.scalar.dma_start(haf0[:], ha0_in[:])
            staged = {}
            for hx in range(split):
                ha_sb, _, _ = new_ha_buffers(0, hx, with_agi=False)
                for m in range(MT):
                    produce_ha(0, hx, m, ha_sb, None, None)

            # the last step's AG+GEMM would only produce p_n, never read
            for k in range(nsteps - 1):
                for hx in range(split):
                    cs = hx * BS
                    if k == 0:
                        haf = haf0[:, :, cs:cs + BS]
                    else:
                        ag_i = staged[hx]
                        ag_o = dram_pool.tile([H, BS], mybir.dt.float8e4,
                                              tag=f"ago_{k}_{hx}", name=f"ago_{k}_{hx}",
                                              addr_space="Shared")
                        nc.gpsimd.collective_compute(
                            "AllGather", mybir.AluOpType.bypass,
                            replica_groups=[list(range(N_CORES))],
                            ins=[ag_i[:].opt()],
                            outs=[ag_o[:].opt()],
                        )
                        haf_t = pool.tile([128, KCH, BS], mybir.dt.float16,
                                          tag=f"haf{hx}", bufs=5, name=f"haf_{k}_{hx}")
                        for g in range(KCH // haf_group):
                            # SWDGE cast fp8 -> fp16 during readback
                            nc.gpsimd.dma_start(
                                haf_t[:, g * haf_group:(g + 1) * haf_group, :],
                                ag_o[g * haf_group * 128:(g + 1) * haf_group * 128, :]
                                   .rearrange("(c p) b -> p c b", p=128),
                            )
                        haf = haf_t[:]
                    ps = {m: psum_pool.tile([128, BS], mybir.dt.float32,
                                            tag=f"ps{hx}_{m}", bufs=1,
                                            name=f"ps_{k}_{hx}_{m}")
                          for m in range(MT)}
                    need_agi = k + 1 <= nsteps - 2
                    ha_next, ha8_next, agi_next = new_ha_buffers(
                        k + 1, hx, with_agi=need_agi)
                    # unit order: m0/m1 finish early (early next-AG payload)
                    # while readback group 3 is first needed ~halfway through
                    UNITS = [(0, 0), (0, 1), (0, 2), (1, 0), (1, 1), (1, 2),
                             (0, 3), (1, 3), (2, 0), (2, 1), (2, 2), (2, 3),
                             (3, 0), (3, 1), (3, 2), (3, 3)]
                    GRP = KCH // MT
                    m_units = {m: 0 for m in range(MT)}
                    for (m, g) in UNITS:
                        for kk in range(GRP * g, GRP * g + GRP):
                            nc.tensor.matmul(
                                ps[m][:],
                                G_sb[:, kk, m * 128:(m + 1) * 128],
                                haf[:, kk, :],
                                start=(m_units[m] == 0 and kk == GRP * g),
                                stop=(m_units[m] == MT - 1
                                      and kk == GRP * g + GRP - 1),
                            )
                        m_units[m] += 1
                        if m_units[m] < MT:
                            continue
                        pupd = nc.vector.tensor_tensor(
                            p_sb[:, m, cs:cs + BS], p_sb[:, m, cs:cs + BS],
                            ps[m][:], mybir.AluOpType.add,
                        )
                        if k == nsteps * 3 // 4 and hx == 0 and m == 0:
                            anchors.append(pupd.ins)
                        produce_ha(k + 1, hx, m, ha_next, ha8_next, agi_next)
                    staged[hx] = agi_next

            from concourse.tile import add_dep_helper
            w2_sb = pool.tile([128, MT, D], mybir.dt.float32r, tag="w2_sb")
            for m in range(MT):
                w2dma = nc.scalar.dma_start(w2_sb[:, m, :], w2_in[m])
                if anchors:
                    add_dep_helper(anchors[0], w2dma.ins, sync=False,
                                   reason="load w2 late")
            S_r = pool.tile([128, MT, B], mybir.dt.float32r, tag="S_r")
            nc.vector.tensor_copy(S_r[:], S_sb[:])
            for mt in range(D // 128):
                psf = psum_pool.tile([128, B], mybir.dt.float32,
                                     tag=f"ps0_{mt % 4}", bufs=1, name=f"psf_{mt}")
                for kk in range(MT):
                    nc.tensor.matmul(
                        psf[:],
                        w2_sb[:, kk, mt * 128:(mt + 1) * 128],
                        S_r[:, kk, :],
                        start=(kk == 0), stop=(kk == MT - 1),
                    )
                zf_sb = pool.tile([128, B], mybir.dt.float32,
                                  tag=f"zf_sb{mt % 4}", bufs=1, name=f"zf_sb_{mt}")
                nc.vector.tensor_copy(zf_sb[:], psf[:])
                nc.sync.dma_start(zf_out[mt], zf_sb[:])

    nc.compile()
    return nc


_PROGRAM_CACHE = {}


def kernel(z0, t, W1, b1, u, W2, b2):
    from concourse.bass_utils import run_bass_kernel_spmd

    z0 = np.asarray(z0)
    t = np.asarray(t)
    W1 = np.asarray(W1)
    b1 = np.asarray(b1)
    u = np.asarray(u)
    W2 = np.asarray(W2)
    b2 = np.asarray(b2)

    sched = _compute_schedule(t)
    if not sched:
        return z0.astype(np.float32).copy()

    key = tuple(sched)
    nc = _PROGRAM_CACHE.get(key)
    if nc is None:
        nc = _build_program(sched)
        _PROGRAM_CACHE[key] = nc
    in_maps = _host_prepare(z0, W1, b1, u, W2, b2, sched)
    res = run_bass_kernel_spmd(nc, in_maps, list(range(N_CORES)))

    f32 = np.float32
    acc = np.zeros((D, B), dtype=f32)
    for r in res.results:
        acc += r["zf_out"].reshape(D, B)
    sumh = f32(np.sum(np.array([h for h, _ in sched], dtype=f32), dtype=np.float64))
    out = z0.astype(f32) + acc.T + sumh * b2.astype(f32)
    return out.astype(np.float32)

